# revision 75
# baseline (speedup 1.0000x reference)
"""Trainium2 Bass kernel for a CrossAttentionBlock (GroupNorm + 8-head
cross-attention + output projection + residual).

Sharding: one attention head per NeuronCore (8 heads / 8 cores).  Each core
computes its head's partial output projection wo[:, h] @ attn_h; the host sums
the 8 partials and adds the residual, output bias and folded V-bias
(wo @ bv rides on sum(attn)=1).

Design (v2 — Activation-engine-roofline schedule):
 - The per-core floor is exp on the ScalarE/Activation engine: 4096x4096
   scores / 128 lanes * 0.83 ns = ~109 us.  Everything else is scheduled to
   hide under a saturated exp pipeline.
 - GroupNorm folds into the Q projection weights (q = (wq*a) @ x + qbias).
   x streams in 8 column blocks with bn_stats chasing each block so the
   stats tail after the last DMA is short.
 - context streams in 8 column blocks of 512 positions; each block is
   immediately KV-projected (f32r, fused k|v output), v is downcast to fp8
   and transposed on the PE (fp8 transposes run 2x f32 ones).
 - Scores run transposed (T[j,i] = k_j . q_i) in f32r, exp on ScalarE with
   the 1/sqrt(dh) scale folded into the activation, output in fp8e4.
 - AV uses fp8e4 DoubleRow matmuls: one matmul contracts a PAIR of key
   tiles (vT_aug[:, j:j+2, :] x ee[:, 0:2, :]) at 0.5 cyc/row.  The softmax
   denominator rides as a ones-column in the augmented V^T.
 - Exp emission runs a 3-phase diagonal: chunks 0-2 sweep key blocks as they
   arrive (phases A/B), then chunks 3-7 run with all keys resident (phase C).
   3 chunks in flight = 3 PSUM accumulator banks (pst 4 + pav 3 + tmp 1 = 8).

Self-contained: hardcodes all shapes from the problem spec.
"""

import sys

sys.path.insert(0, "/opt/trn_rl_repo")

import numpy as np

import concourse.bass as bass
import concourse.tile as tile
from concourse import bacc, mybir

F32 = mybir.dt.float32
F32R = mybir.dt.float32r
BF16 = mybir.dt.bfloat16
FP8 = mybir.dt.float8e4

CH = 512          # x channels
CTXC = 768        # context channels
N = 4096          # spatial positions (64*64)
NH = 8            # heads
DH = 64           # head dim
G = 32            # groupnorm groups
EPS = 1e-5
NCO = CH // 128   # x channel blocks (4)
NCK = CTXC // 128  # ctx channel blocks (6)
IC = 512          # query-chunk size
NIC = N // IC     # 8 query chunks
NJT = N // 128    # 32 key tiles
NKB = 8           # ctx column blocks (512 positions each)
KBW = N // NKB    # 512
SCALE = 1.0 / 8.0  # 1/sqrt(DH), applied inside the exp activation

ADD = mybir.AluOpType.add
SUB = mybir.AluOpType.subtract
MUL = mybir.AluOpType.mult
DR = mybir.MatmulPerfMode.DoubleRow

# packed-weights column offsets (f32 columns of a [128, WPACK] tensor)
O_WQT = 0                 # [128, 4, 64]   wqT rearranged (co p) d -> p co d
O_WKVT = 256              # [128, 6, 128]  wkvT rearranged
O_WOT = 1024              # [64, 512]      woT (rows 0:64)
O_GNW = 1536              # [128, 4]
O_GNB = 1540              # [128, 4]
O_GMAT = 1544             # [128, 8]
O_GMATT = 1552            # [8, 128]
O_IDENT = 1680            # [128, 128]
O_BQ = 1808               # [64, 1]
O_BK = 1809               # [64, 1]
WPACK = 1810


def build_nc():
    nc = bacc.Bacc("TRN2", num_devices=8, debug=False)

    x = nc.dram_tensor("x", (CH, N), F32R, kind="ExternalInput")
    ctx_t = nc.dram_tensor("ctx", (CTXC, N), F32R, kind="ExternalInput")
    # all weights/constants packed into one [128, WPACK] tensor: a single DMA
    # avoids ~6us of per-transfer HWDGE overhead on the critical startup path
    wpack_d = nc.dram_tensor("wpack", (128, WPACK), F32, kind="ExternalInput")
    partial = nc.dram_tensor("partial", (CH, N), F32, kind="ExternalOutput")

    xv = x.rearrange("(co p) n -> p co n", p=128)
    ctxv = ctx_t.rearrange("(ck p) n -> p ck n", p=128)
    pvw = partial.rearrange("(co p) n -> p co n", p=128)

    with tile.TileContext(nc) as tc:
        singles = tc.alloc_tile_pool(name="singles", bufs=1)
        # PSUM budget: pst 2x2 + pav 2x1 + den 1x1 + bsh 1x1 = 8 banks
        pst = tc.alloc_tile_pool(name="pst", bufs=2, space="PSUM")
        psav = tc.alloc_tile_pool(name="psav", bufs=2, space="PSUM")
        psden = tc.alloc_tile_pool(name="psden", bufs=1, space="PSUM")
        bsh = tc.alloc_tile_pool(name="bsh", bufs=1, space="PSUM")
        # ee tiles are retained per chunk for the batched denominator matmuls
        expp = tc.alloc_tile_pool(name="expp", bufs=34)
        attp = tc.alloc_tile_pool(name="attp", bufs=2)
        outp = tc.alloc_tile_pool(name="outp", bufs=4)
        ctxq = tc.alloc_tile_pool(name="ctxq", bufs=3)
        vrp = tc.alloc_tile_pool(name="vrp", bufs=2)

        # ---------------- persistent SBUF tiles ----------------
        x_sb = singles.tile([128, NCO, N], F32R)
        q_sb = singles.tile([DH, N], F32R)
        k_sb = singles.tile([DH, N], F32R)
        vT_sb = singles.tile([128, NJT, DH], FP8)
        ones8 = singles.tile([128, 2, 32], FP8)  # DoubleRow denominator weights
        wpack_sb = singles.tile([128, WPACK], F32)
        wqT_sb = wpack_sb[:, O_WQT:O_WQT + 256].rearrange("p (co d) -> p co d", d=DH)
        wkvT_sb = wpack_sb[:, O_WKVT:O_WKVT + 768].rearrange("p (ck d) -> p ck d",
                                                             d=2 * DH)
        woT_sb = wpack_sb[0:DH, O_WOT:O_WOT + CH]
        gmat_sb = wpack_sb[:, O_GMAT:O_GMAT + 8]
        gmatT_sb = wpack_sb[0:8, O_GMATT:O_GMATT + 128]
        ident_sb = wpack_sb[:, O_IDENT:O_IDENT + 128]
        bq_sb = wpack_sb[0:DH, O_BQ:O_BQ + 1]
        kb_sb = wpack_sb[0:DH, O_BK:O_BK + 1]
        wqs_sb = singles.tile([128, NCO, DH], F32R)
        wkv_r = singles.tile([128, NCK, 2 * DH], F32R)
        woT_r = singles.tile([DH, CH], F32R)
        ones1 = singles.tile([1, DH], F32R)
        st_all = singles.tile([128, NCO, 2 * NKB, 6], F32)
        mv3 = singles.tile([128, NCO, 3], F32)
        gsm = singles.tile([8, NCO, 3], F32)
        gmu84 = singles.tile([8, NCO], F32)
        gvar84 = singles.tile([8, NCO], F32)
        srt84 = singles.tile([8, NCO], F32)
        grs = singles.tile([8, NCO, 2], F32)
        rg_pc = singles.tile([128, NCO, 2], F32)
        tmp_pc = singles.tile([128, NCO], F32)
        qbias = singles.tile([DH, 1], F32)


        # ---------------- phase 1: x column blocks + stats ----------------
        # x first: it is on the critical path to the q projection, and its
        # per-block stats chase the DMAs so the tail after the last block is
        # one bn_stats round (256-column granules keep that tail short).
        XB = 2 * NKB
        XBW = N // XB
        for b in range(XB):
            sl = slice(b * XBW, (b + 1) * XBW)
            nc.sync.dma_start(out=x_sb[:, :, sl], in_=xv[:, :, sl])
            for co in range(NCO):
                nc.vector.bn_stats(out=st_all[:, co, b, :],
                                   in_=x_sb[:, co, sl].bitcast(F32))

        # ---------------- phase 2: packed weight load ----------------
        nc.sync.dma_start(out=wpack_sb[:], in_=wpack_d[:])

        # constants
        onesf = singles.tile([128, 64], F32)
        nc.vector.memset(onesf[:], 1.0)
        nc.vector.tensor_copy(out=ones1[:], in_=onesf[0:1, 0:DH])
        nc.vector.memset(ones8[:], 1.0)
        # wkv_r prep (needed by k_proj(0) at ~31us; emitted before the
        # aggregation chain so the DVE FIFO has it early)
        nc.vector.tensor_copy(out=wkv_r[:], in_=wkvT_sb[:])

        # ---------------- phase 3: ctx block 0 DMA (two halves so the
        # first key tiles land ~2us earlier) ----------------
        ctx_tiles = []
        v_tiles = []
        ct0 = ctxq.tile([128, NCK, KBW], F32R, tag="ctq", name="ctq")
        nc.sync.dma_start(out=ct0[:, :, 0:KBW // 2], in_=ctxv[:, :, 0:KBW // 2])
        nc.sync.dma_start(out=ct0[:, :, KBW // 2:KBW],
                          in_=ctxv[:, :, KBW // 2:KBW])
        ctx_tiles.append(ct0)

        # ---------------- groupnorm aggregation ----------------
        # gn_w is folded into wqT host-side and wq@gn_b + bq arrives as the
        # packed qbias base, so the device chain reduces to:
        #   group (mean, var) -> rstd -> per-channel rstd -> wqs / qbias
        for co in range(NCO):
            nc.vector.bn_aggr(out=mv3[:, co, 0:2], in_=st_all[:, co, :, :])
        nc.vector.tensor_tensor(out=mv3[:, :, 2], in0=mv3[:, :, 0],
                                in1=mv3[:, :, 0], op=MUL)
        # per-group sums of (mean, var, mean^2) via tiny PE matmuls
        psg = pst.tile([8, NCO, 3], F32, tag="tt", name="psg")
        for co in range(NCO):
            nc.tensor.matmul(psg[:, co, :], gmat_sb[:], mv3[:, co, :],
                             start=(co == 0), stop=(co == NCO - 1))
        nc.vector.tensor_copy(out=gsm[:], in_=psg[:])
        # var_g = (sum var + sum mean^2)/16 - mu_g^2  (eps dropped: var ~ 1)
        nc.vector.tensor_scalar_mul(out=gmu84[:], in0=gsm[:, :, 0], scalar1=1.0 / 16.0)
        nc.vector.tensor_tensor(out=srt84[:], in0=gmu84[:], in1=gmu84[:], op=MUL)
        nc.vector.tensor_tensor(out=gvar84[:], in0=gsm[:, :, 1], in1=gsm[:, :, 2],
                                op=ADD)
        nc.vector.scalar_tensor_tensor(out=gvar84[:], in0=gvar84[:],
                                       scalar=1.0 / 16.0, in1=srt84[:],
                                       op0=MUL, op1=SUB)
        with nc.allow_low_precision(reason="rstd via approx reciprocal"):
            nc.vector.reciprocal(out=srt84[:], in_=gvar84[:])
        nc.scalar.activation(out=grs[:, :, 0], in_=srt84[:],
                             func=mybir.ActivationFunctionType.Sqrt)
        nc.vector.tensor_copy(out=grs[:, :, 1], in_=gmu84[:])
        psr = pst.tile([128, NCO, 2], F32, tag="tt", name="psr")
        for co in range(NCO):
            nc.tensor.matmul(psr[:, co, :], gmatT_sb[:], grs[:, co, :],
                             start=(co == 0), stop=(co == NCO - 1))
        nc.vector.tensor_copy(out=rg_pc[:], in_=psr[:])
        # wqs = (wq*gnw)T * rstd ; qb-term = (wq*gnw)T @ (mu*rstd)
        for co in range(NCO):
            nc.vector.tensor_scalar_mul(out=wqs_sb[:, co, :], in0=wqT_sb[:, co, :],
                                        scalar1=rg_pc[:, co, 0:1])
        nc.vector.tensor_tensor(out=tmp_pc[:], in0=rg_pc[:, :, 0],
                                in1=rg_pc[:, :, 1], op=MUL)
        qb = pst.tile([DH, 1], F32, tag="tt", name="qb")
        for co in range(NCO):
            nc.tensor.matmul(qb[:], wqT_sb[:, co, :], tmp_pc[:, co:co + 1],
                             start=(co == 0), stop=(co == NCO - 1))
        nc.vector.tensor_tensor(out=qbias[:], in0=bq_sb[:], in1=qb[:], op=SUB)

        # ---------------- building blocks ----------------
        def ctx_dma(b):
            # two half-DMAs per block: the first key tiles of each block are
            # projectable half a DMA period earlier
            ct = ctxq.tile([128, NCK, KBW], F32R, tag="ctq", name="ctq")
            o = b * KBW
            nc.sync.dma_start(out=ct[:, :, 0:KBW // 2],
                              in_=ctxv[:, :, o:o + KBW // 2])
            nc.sync.dma_start(out=ct[:, :, KBW // 2:KBW],
                              in_=ctxv[:, :, o + KBW // 2:o + KBW])
            ctx_tiles.append(ct)

        def q_proj(ic):
            """q chunk ic -> q_sb (unscaled; 1/sqrt(dh) rides in the exp)."""
            sl = slice(ic * IC, (ic + 1) * IC)
            pq = bsh.tile([DH, IC], F32, tag="b", name="pq")
            for co in range(NCO):
                nc.tensor.matmul(pq[:], wqs_sb[:, co, :], x_sb[:, co, sl],
                                 start=(co == 0), stop=(co == NCO - 1))
            nc.vector.tensor_scalar(out=q_sb[:, sl], in0=pq[:],
                                    scalar1=qbias[:], scalar2=None, op0=ADD)

        def k_proj(b, halves=1):
            """fused k|v projection for ctx block b -> k_sb + v ring tile.

            pkv tiles ride the den pool ring (dens only exist after the last
            kv block, so the ring never sees both at once); this keeps the
            b-ring free for pq/pvt/pb/po and off the kv critical path.
            halves=2 projects the block in two column halves so the first
            key tiles are ready one half-DMA earlier (block 0 only)."""
            vr = vrp.tile([128, KBW], F32, tag="vr", name="vr")
            ct = ctx_tiles[b]
            hw = KBW // halves
            for h in range(halves):
                hsl = slice(h * hw, (h + 1) * hw)
                sl = slice(b * KBW + h * hw, b * KBW + (h + 1) * hw)
                pkv = psden.tile([128, hw], F32, tag="den", name="pkv")
                for ck in range(NCK):
                    nc.tensor.matmul(pkv[:], wkv_r[:, ck, :], ct[:, ck, hsl],
                                     start=(ck == 0), stop=(ck == NCK - 1))
                nc.vector.tensor_scalar(out=k_sb[:, sl], in0=pkv[0:64, :],
                                        scalar1=kb_sb[:], scalar2=None, op0=ADD)
                nc.vector.tensor_copy(out=vr[64:128, hsl], in_=pkv[64:128, :])
            v_tiles.append(vr)

        def vT_proj(b):
            """transpose v for ctx block b -> vT_sb[:, 4b:4b+4, :].

            All 4 transposes land in one psum tile; a single fp8 copy drains
            it (one ring slot round-trip instead of four)."""
            vr = v_tiles[b]
            pvt = bsh.tile([128, 4, DH], F32, tag="b", name="pvt")
            for t in range(4):
                # one accumulation group over disjoint columns: a start=True
                # per slice would re-zero the whole 2KB psum zero-region
                nc.tensor.matmul(pvt[:, t, :],
                                 vr[64:128, t * 128:(t + 1) * 128],
                                 ident_sb[64:128, 64:128],
                                 is_transpose=True,
                                 start=(t == 0), stop=(t == 3))
            with nc.allow_low_precision(reason="fp8 AV operand"):
                nc.vector.tensor_copy(out=vT_sb[:, 4 * b:4 * b + 4, :], in_=pvt[:])

        pav_tiles = {}
        ee_tiles = {}

        def qk_part(ic, jp):
            """QK + exp for chunk ic, key-tile pair jp."""
            sl = slice(ic * IC, (ic + 1) * IC)
            if jp == 0:
                pav_tiles[ic] = psav.tile([DH, IC], F32, tag="pav", name="pav")
                ee_tiles[ic] = []
            jA, jB = 2 * jp, 2 * jp + 1
            tt = pst.tile([128, 2, IC], F32, tag="tt", name="tps")
            nc.tensor.matmul(tt[:, 0, :], k_sb[:, jA * 128:(jA + 1) * 128],
                             q_sb[:, sl], start=True, stop=True)
            nc.tensor.matmul(tt[:, 1, :], k_sb[:, jB * 128:(jB + 1) * 128],
                             q_sb[:, sl], start=True, stop=True)
            ee = expp.tile([128, 2, IC], FP8, tag="exp", name="exp")
            nc.scalar.activation(out=ee[:], in_=tt[:],
                                 func=mybir.ActivationFunctionType.Exp,
                                 scale=SCALE)
            ee_tiles[ic].append(ee)

        def av_part(ic, jp):
            pav = pav_tiles[ic]
            jA = 2 * jp
            ee = ee_tiles[ic][jp]
            nc.tensor.matmul(pav[:], vT_sb[:, jA:jA + 2, :], ee[:],
                             start=(jp == 0), stop=(jp == NJT // 2 - 1),
                             perf_mode=DR)

        def att_pair(ic, jp):
            qk_part(ic, jp)
            av_part(ic, jp)

        den_tiles = {}
        rden_tiles = {}

        def att_den_mms(ic, lo, hi):
            """denominator matmuls for pairs [lo, hi) of chunk ic."""
            if lo == 0:
                den_tiles[ic] = psden.tile([32, IC], F32, tag="den", name="den")
            den = den_tiles[ic]
            ees = ee_tiles[ic]
            for i in range(lo, hi):
                nc.tensor.matmul(den[:], ones8[:], ees[i][:],
                                 start=(i == 0), stop=(i == NJT // 2 - 1),
                                 perf_mode=DR)
            if hi == NJT // 2:
                ee_tiles.pop(ic)
                rden = attp.tile([1, IC], F32R, tag="rden", name="rden")
                with nc.allow_low_precision(reason="f32r matmul operand"):
                    nc.vector.reciprocal(out=rden[:], in_=den_tiles.pop(ic)[0:1, :])
                rden_tiles[ic] = rden

        def att_norm(ic):
            """rden broadcast + attn normalize (straight off the psum pb)."""
            pav = pav_tiles.pop(ic)
            rden = rden_tiles.pop(ic)
            pb = bsh.tile([DH, IC], F32, tag="b", name="pb")
            nc.tensor.matmul(pb[:], ones1[:], rden[:], start=True, stop=True)
            rb = attp.tile([DH, IC], F32, tag="rb", name="rb")
            nc.vector.tensor_copy(out=rb[:], in_=pb[:])
            attn = attp.tile([DH, IC], F32R, tag="attn", name="attn")
            nc.vector.tensor_tensor(out=attn[:], in0=pav[:], in1=rb[:], op=MUL)
            return attn

        def att_store(ic, attn, ot, fast=False):
            sl = slice(ic * IC, (ic + 1) * IC)
            if fast:
                po = pst.tile([128, IC], F32, tag="tt", name="po")
            else:
                po = bsh.tile([128, IC], F32, tag="b", name="po")
            nc.tensor.matmul(po[:], woT_r[:, ot * 128:(ot + 1) * 128],
                             attn[:], start=True, stop=True)
            pt = outp.tile([128, IC], F32, tag="pt", name="pt")
            if fast and ot % 2 == 1:
                # ScalarE is idle after the last exp; split the drain copies
                nc.scalar.copy(out=pt[:], in_=po[:])
            else:
                nc.vector.tensor_copy(out=pt[:], in_=po[:])
            nc.sync.dma_start(out=pvw[:, ot, sl], in_=pt[:])

        # ---------------- emission schedule ----------------
        # DMA order on SP: weights, x blocks, ctx blocks in stream order,
        # then the output stores (inside att_finish).  Chunks 0-1 sweep the
        # key blocks diagonally as they arrive (2 PSUM accumulators), then
        # chunks 2-7 run with all keys resident.  Each chunk's finish is
        # emitted behind the next chunk's first pairs so the PE-side finish
        # work (den/pb/po) never delays the exp stream.
        ctx_dma(1)
        ctx_dma(2)
        # deprioritized: the scheduler must not run these ahead of the tiny
        # groupnorm-chain matmuls that gate the q projection
        with tc.high_priority(offset=-4000):
            k_proj(0, halves=2)
        q_proj(0)
        qk_part(0, 0)        # first exps fill both tt slots from chunk 0
        qk_part(0, 1)
        q_proj(1)
        qk_part(1, 0)
        qk_part(1, 1)
        vT_proj(0)
        av_part(0, 0)
        av_part(0, 1)
        av_part(1, 0)
        av_part(1, 1)
        with tc.high_priority(offset=-4000):
            k_proj(1)
        vT_proj(1)
        q_proj(2)
        nc.vector.tensor_copy(out=woT_r[:], in_=woT_sb[:])

        # streaming phase: chunks 0-1 sweep key blocks as they arrive.  The
        # next block's kv work is emitted right after the current block's
        # FIRST pair so the PE sequencer reaches it while the remaining
        # (exp-paced) pairs are still draining — by then its ctx DMA has
        # landed and k(b+1) is ready one DMA period after k(b).
        for b in range(1, NKB):
            if b + 2 < NKB:
                ctx_dma(b + 2)
            att_pair(0, 2 * b)
            att_pair(1, 2 * b)
            if b + 1 < NKB:
                k_proj(b + 1, halves=2)
            att_pair(0, 2 * b + 1)
            att_pair(1, 2 * b + 1)
            if b + 1 < NKB:
                vT_proj(b + 1)

        att_den_mms(0, 0, NJT // 2)
        attn0 = att_norm(0)
        for ot in range(NCO):
            att_store(0, attn0, ot)
        nq = 3
        NP = NJT // 2
        for c in range(2, NIC):
            last = c == NIC - 1
            att_pair(c, 0)
            att_pair(c, 1)
            att_pair(c, 2)
            att_pair(c, 3)
            att_den_mms(c - 1, 0, 8)
            att_pair(c, 4)
            att_pair(c, 5)
            att_den_mms(c - 1, 8, NP)
            att_pair(c, 6)
            att_pair(c, 7)
            attn = att_norm(c - 1)
            for jp in range(8, NP):
                att_pair(c, jp)
                if jp - 8 < NCO:
                    att_store(c - 1, attn, jp - 8)
                if jp == 12 and nq < NIC:
                    q_proj(nq)
                    nq += 1
                # interleave the LAST chunk's denominator behind its own
                # pairs so the final finish chain is short
                if last and jp >= NP - 3:
                    att_den_mms(c, 4 * (jp - NP + 3), 4 * (jp - NP + 4))
        att_den_mms(NIC - 1, 12, NP)
        attn = att_norm(NIC - 1)
        for ot in range(NCO):
            att_store(NIC - 1, attn, ot, fast=True)

        for p in [vrp, ctxq, outp, attp, expp, bsh, psden, psav, pst, singles]:
            p.release()

    nc.compile()
    return nc


GMAT = (np.arange(128)[:, None] // 16 == np.arange(8)[None, :]).astype(np.float32)
IDENT = np.eye(128, dtype=np.float32)


def _pack_weights(wq, wkvT_h, woT_h, gn_w, gn_b, bq_h, bk_h, hs):
    wp = np.zeros((128, WPACK), dtype=np.float32)
    # gn_w folds into the q weights; wq@gn_b + bq becomes the qbias base
    wqT_h = (wq[hs, :] * gn_w[None, :]).T  # [512, 64]
    wp[:, O_WQT:O_WQT + 256] = wqT_h.reshape(NCO, 128, DH).transpose(1, 0, 2) \
        .reshape(128, 256)
    wp[:, O_WKVT:O_WKVT + 768] = wkvT_h.reshape(NCK, 128, 2 * DH) \
        .transpose(1, 0, 2).reshape(128, 768)
    wp[0:DH, O_WOT:O_WOT + CH] = woT_h
    wp[:, O_GMAT:O_GMAT + 8] = GMAT
    wp[0:8, O_GMATT:O_GMATT + 128] = GMAT.T
    wp[:, O_IDENT:O_IDENT + 128] = IDENT
    wp[0:DH, O_BQ] = wq[hs, :] @ gn_b + bq_h
    wp[0:DH, O_BK] = bk_h
    return wp

_NC_CACHE = None


def get_nc():
    global _NC_CACHE
    if _NC_CACHE is None:
        _NC_CACHE = build_nc()
    return _NC_CACHE


def kernel(x, context, gn_w, gn_b, wq, bq, wk, bk, wv, bv, wo, bo):
    from concourse.bass_utils import run_bass_kernel_spmd

    x = np.asarray(x, dtype=np.float32)
    context = np.asarray(context, dtype=np.float32)
    gn_w = np.asarray(gn_w, dtype=np.float32)
    gn_b = np.asarray(gn_b, dtype=np.float32)
    wq = np.asarray(wq, dtype=np.float32)
    bq = np.asarray(bq, dtype=np.float32)
    wk = np.asarray(wk, dtype=np.float32)
    bk = np.asarray(bk, dtype=np.float32)
    wv = np.asarray(wv, dtype=np.float32)
    bv = np.asarray(bv, dtype=np.float32)
    wo = np.asarray(wo, dtype=np.float32)
    bo = np.asarray(bo, dtype=np.float32)

    B, C, H, W = x.shape
    x2 = np.ascontiguousarray(x.reshape(C, H * W))
    ctx2 = np.ascontiguousarray(context.reshape(CTXC, H * W))

    in_maps = []
    for h in range(NH):
        hs = slice(h * DH, (h + 1) * DH)
        wkvT_h = np.concatenate([wk[hs, :].T, wv[hs, :].T], axis=1)
        wp = _pack_weights(wq, wkvT_h, wo[:, hs].T, gn_w, gn_b,
                           bq[hs], bk[hs], hs)
        in_maps.append({
            "x": x2,
            "ctx": ctx2,
            "wpack": wp,
        })

    nc = get_nc()
    res = run_bass_kernel_spmd(nc, in_maps, core_ids=list(range(NH)))
    acc = np.zeros((C, H * W), dtype=np.float64)
    for h in range(NH):
        acc += res.results[h]["partial"]
    # residual, output bias, and the folded V bias (sum(attn) == 1) are added
    # host-side during the unshard.
    acc += x2
    acc += (bo + wo @ bv)[:, None]
    return acc.astype(np.float32).reshape(B, C, H, W)
